# revision 49
# baseline (speedup 1.0000x reference)
"""EncDec ConvLSTM kernel for 8 Trainium2 NeuronCores.

Sharding: 8 cores = 4 (batch) x 2 (spatial row-halves). Each core computes
its 32 output rows plus a shrinking redundant halo (21-s extra rows at
recurrent step s), so no cross-core communication is needed. Row-half 1
cores receive a vertically flipped image and ky-flipped conv weights, so a
single SPMD program serves all cores.

Conv3x3 is mapped to PE matmuls over pixels (N=512 free dim, bf16).
State tile R[128, grid]: partitions 0:64 hold h, partitions 64:128 hold h
col-shifted by +2 (one SBUF->SBUF DMA per tile, off the critical path).
Per 8-row tile and M-tile: 1 x-im2col MM (K=72), 3 paired-kx MMs (K=128,
reading [h | h+2col] at row offsets 0/1/2), and 3 middle-column taps as
K=64 row-strip MMs reading the same tile: ky=0,2 from the lower half at
col offset +1, ky=1 from the upper half at col offset -1 (the col-shifted
copy re-read one col left IS the middle column). No ta/tb packed copies.
The sig(i)*tanh(g) product is written cross-partition (in@64:128 ->
out@0:63) directly by the DVE, eliminating the old t1l DMA.
"""

import os
import sys

import numpy as np

for _p in ("/opt/trn_rl_repo", "/root/.axon_site/_ro/trn_rl_repo"):
    if os.path.isdir(_p) and _p not in sys.path:
        sys.path.append(_p)

T = 10
F = 8
HD = 64
HS = 64
WS = 64
NCORES = 8
PW = 66   # padded grid width/height
LEAD = 66  # one extra leading pad row in the R state tile
RSZ = LEAD + PW * PW + 2  # flat elems per partition in R
NSTEPS = 2 * T

_CACHE = {}


def _regions():
    """Rounded compute-region row counts per recurrent step s=1..NSTEPS."""
    out = []
    for s in range(1, NSTEPS + 1):
        need = NSTEPS + 1 - s
        rows = min(HS, 32 + need)
        rows = min(HS, ((rows + 7) // 8) * 8)
        out.append(rows)
    return out


def _build_program(use_bf16=True):
    from concourse import bacc, mybir, tile

    F32 = mybir.dt.float32
    MMDT = mybir.dt.bfloat16 if use_bf16 else mybir.dt.float32r
    ACT = mybir.ActivationFunctionType

    nc = bacc.Bacc("TRN2", target_bir_lowering=False, debug=False,
                   num_devices=NCORES)

    def din(name, shape, dt=MMDT):
        return nc.dram_tensor(name, shape, dt, kind="ExternalInput").ap()

    xe_d = din("xe", [T, F, PW, PW])
    xd_d = din("xd", [T, F, PW, PW])
    w_x = {"e": din("w_ex", [128, 256]), "d": din("w_dx", [128, 256])}
    w_p = {ph: [din(f"w_{ph}p{k}", [128, 256]) for k in range(3)]
           for ph in ("e", "d")}
    # middle-column (kx=1) taps: mA = [ky0; ky1] (K=128 on R2), mB = [ky2]
    # (mB/x weights zero-padded to K=128: full-row LDWEIGHTS can hide
    # behind in-flight matmuls, partial-row ones cannot)
    w_ma = {ph: din(f"w_{ph}ma", [128, 256]) for ph in ("e", "d")}
    w_mb = {ph: din(f"w_{ph}mb", [128, 256]) for ph in ("e", "d")}
    w_op = [din(f"w_op{k}", [128, 8]) for k in range(3)]
    w_oma = din("w_oma", [128, 8])
    w_omb = din("w_omb", [128, 8])
    scl_d = din("scl", [128, 1], F32)  # og tanh scale: 0.5 (o) / 1.0 (g)
    use_xbase = os.environ.get("KERNEL_XBASE", "1") == "1"
    s1_skip = os.environ.get("KERNEL_S1SKIP", "1") == "1"
    ALU = mybir.AluOpType
    b_m0 = {"e": din("b_e0", [128, 1], F32), "d": din("b_d0", [128, 1], F32)}
    b_m1 = {"e": din("b_e1", [128, 1], F32), "d": din("b_d1", [128, 1], F32)}
    b_o = din("b_o", [8, 1], F32)
    y_d = nc.dram_tensor("y", [T, F, 32, WS], F32, kind="ExternalOutput").ap()

    regions = _regions()

    with tile.TileContext(nc) as tc:
        with tc.tile_pool(name="wpool", bufs=1) as wp, \
             tc.tile_pool(name="state", bufs=1) as stp, \
             tc.tile_pool(name="x2p", bufs=2) as x2p, \
             tc.tile_pool(name="gps", bufs=6, space="PSUM") as gps, \
             tc.tile_pool(name="ops", bufs=2, space="PSUM") as ops, \
             tc.tile_pool(name="fip", bufs=3) as fip, \
             tc.tile_pool(name="ogp", bufs=3) as ogp, \
             tc.tile_pool(name="t0p", bufs=3) as t0p, \
             tc.tile_pool(name="t1p", bufs=3) as t1p, \
             tc.tile_pool(name="thp", bufs=3) as thp, \
             tc.tile_pool(name="yyp", bufs=2) as yyp:

            # ---- load weights / biases into SBUF ----
            # Issue order = sync-queue drain order: encoder first, then
            # out-conv, decoder last (not needed until step T+1).
            def wtile(src, shape, tag, dt=MMDT):
                t_ = wp.tile(shape, dt, tag=tag)
                nc.sync.dma_start(t_[:], src[:])
                return t_

            sw_x, sw_p, sw_ma, sw_mb, sb_m0, sb_m1 = {}, {}, {}, {}, {}, {}
            for ph in ("e", "d"):
                sw_x[ph] = wtile(w_x[ph], [128, 256], f"wx{ph}")
                sw_p[ph] = [wtile(w_p[ph][k], [128, 256], f"wp{ph}{k}")
                            for k in range(3)]
                sw_ma[ph] = wtile(w_ma[ph], [128, 256], f"wma{ph}")
                sw_mb[ph] = wtile(w_mb[ph], [128, 256], f"wmb{ph}")
                sb_m0[ph] = wtile(b_m0[ph], [128, 1], f"b0{ph}", F32)
                sb_m1[ph] = wtile(b_m1[ph], [128, 1], f"b1{ph}", F32)
                if ph == "e":
                    sw_op = [wtile(w_op[k], [128, 8], f"wop{k}")
                             for k in range(3)]
                    sw_oma = wtile(w_oma, [128, 8], "woma")
                    sw_omb = wtile(w_omb, [128, 8], "womb")
                    sb_o = wtile(b_o, [8, 1], "bo", F32)
                    sscl = wtile(scl_d, [128, 1], "scl", F32)

            # ---- persistent state ----
            # R:  [h (parts 0:64) | h col-shifted +2 (parts 64:128)]
            # R2: [h (parts 0:64) | h row-shifted +1 (parts 64:128)] --
            # lets the ky=0/ky=1 middle-column taps run as one full-K=128
            # matmul (partial-row LDWEIGHTS can't hide behind in-flight MMs)
            rrA = stp.tile([128, RSZ], MMDT, tag="rrA")
            rrB = stp.tile([128, RSZ], MMDT, tag="rrB")
            r2A = stp.tile([128, RSZ], MMDT, tag="r2A")
            r2B = stp.tile([128, RSZ], MMDT, tag="r2B")
            c_t = stp.tile([64, HS * WS], F32, tag="c")
            nc.vector.memset(rrA[:], 0.0)
            nc.vector.memset(rrB[:], 0.0)
            nc.vector.memset(r2A[:], 0.0)
            nc.vector.memset(r2B[:], 0.0)
            nc.vector.memset(c_t[:], 0.0)

            # PE clock warm-up: ~3.4us of sustained matmul activity keeps
            # the HAM clock gate at 8/8 before the real work starts.
            for _ in range(24):
                wu = ops.tile([8, 512], F32, tag="pso")
                nc.tensor.matmul(wu[:], sw_op[0][:], rrA[0:128, 0:512],
                                 start=True, stop=True)

            def gview(t_, p0, p1, flat_off, nr=8):
                """[p1-p0, nr, 64] view of grid tile at flat elem offset."""
                v = t_[p0:p1, flat_off:flat_off + nr * PW]
                v = v.rearrange("p (r c) -> p r c", c=PW)
                return v[:, 0:nr, 0:64]

            # x im2col double buffer: partitions 72:128 stay zero forever
            # so the x matmul can run as full-K=128 (weight rows 72+ are 0)
            x2bufs = [x2p.tile([128, 57 * PW], MMDT, tag="x2", name=f"x2{i}")
                      for i in range(2)]
            for _x2 in x2bufs:
                nc.vector.memset(_x2[64:128], 0.0)

            def emit_x2col(s):
                """Load x im2col for step s: partition (ky*3+kx)*8+ic holds
                the flat padded image shifted by ky*66+kx (contiguous)."""
                ph = "e" if s <= T else "d"
                t_idx = (s - 1) if ph == "e" else (s - 1 - T)
                x_src = xe_d if ph == "e" else xd_d
                rp = regions[s - 1]
                ln = (rp - 1) * PW + 64
                x2 = x2bufs[s % 2]
                flat = x_src[t_idx].rearrange("a r c -> a (r c)")
                for tap in range(9):
                    sh = (tap // 3) * PW + (tap % 3)
                    nc.gpsimd.dma_start(x2[tap * 8:(tap + 1) * 8, 0:ln],
                                        flat[:, sh:sh + ln])
                return x2

            def gate_mms(ps, wx, wp3, wma, wmb, ms, x2v, R, R2, r0,
                         skip_h):
                """Accumulate all 4H-gate conv taps for one M-tile."""
                nc.tensor.matmul(ps, wx[:, ms],
                                 x2v[0:128, r0:r0 + 8, 0:64],
                                 start=True, stop=skip_h)
                if skip_h:
                    return
                for k in range(3):
                    nc.tensor.matmul(
                        ps, wp3[k][:, ms],
                        gview(R, 0, 128, LEAD + (r0 + k) * PW),
                        start=False, stop=False)
                # middle column (kx=1): ky=0/1 as one K=128 MM on R2,
                # ky=2 as K=64 on the h half of R
                nc.tensor.matmul(ps, wma[:, ms],
                                 gview(R2, 0, 128, LEAD + r0 * PW + 1),
                                 start=False, stop=False)
                nc.tensor.matmul(ps, wmb[:, ms],
                                 gview(R, 0, 128, LEAD + (r0 + 2) * PW + 1),
                                 start=False, stop=True)

            def emit_outconv1(s, R, R2, n2):
                """relu(out conv) for decoder step s, rows 8*n2..8*n2+7."""
                t_o = s - 1 - T
                r0 = n2 * 8
                pso = ops.tile([8, 512], F32, tag="pso")
                for k in range(3):
                    nc.tensor.matmul(pso[:], sw_op[k][:],
                                     gview(R, 0, 128, LEAD + (r0 + k) * PW),
                                     start=(k == 0), stop=False)
                nc.tensor.matmul(pso[:], sw_oma[:, :],
                                 gview(R2, 0, 128, LEAD + r0 * PW + 1),
                                 start=False, stop=False)
                nc.tensor.matmul(pso[:], sw_omb[:, :],
                                 gview(R, 0, 128, LEAD + (r0 + 2) * PW + 1),
                                 start=False, stop=True)
                yy = yyp.tile([8, 512], F32, tag="yy")
                nc.scalar.activation(yy[:], pso[:], ACT.Relu, bias=sb_o[:])
                nc.gpsimd.dma_start(
                    y_d[t_o, :, r0:r0 + 8, :],
                    yy[:].rearrange("p (r c) -> p r c", c=64))

            def gate_block(s, ph, R_r, R2_r, R_w, R2_w, x2v, r0):
                """Gate conv + c-update for one 8-row tile.

                All gate nonlinearities run as tanh (sigmoid(z) =
                0.5*tanh(z/2)+0.5, with the /2 folded into ACT scale/bias
                and the affine fixup into DVE scalar_tensor_tensor ops).
                Stored h and c carry a 2x factor; h-tap weights are
                pre-halved on the host to compensate.
                """
                skip_h = s1_skip and s == 1
                ps0 = gps.tile([128, 512], F32, tag="ps")
                ps1 = gps.tile([128, 512], F32, tag="ps")
                gate_mms(ps0[:], sw_x[ph], sw_p[ph], sw_ma[ph],
                         sw_mb[ph], slice(0, 128), x2v,
                         R_r, R2_r, r0, skip_h)
                gate_mms(ps1[:], sw_x[ph], sw_p[ph], sw_ma[ph],
                         sw_mb[ph], slice(128, 256), x2v,
                         R_r, R2_r, r0, skip_h)

                # epilogue: M0=[f;i] M1=[o;g], all via tanh
                # fi_t = tanh(z/2) for f,i;  og_t = [tanh(zo/2); tanh(zg)]
                fi = fip.tile([128, 512], MMDT, tag="fi")
                og = ogp.tile([128, 512], MMDT, tag="og")
                nc.scalar.activation(fi[:], ps0[:], ACT.Tanh,
                                     bias=sb_m0[ph][:], scale=0.5)
                nc.scalar.activation(og[:], ps1[:], ACT.Tanh,
                                     bias=sb_m1[ph][:], scale=sscl[:])
                cs = c_t[:, r0 * 64:r0 * 64 + 512]
                # c_stored = 2c;  c_new_stored = 0.5*A + B with
                # A = c_stored*(f_t+1) and B = (i_t+1)*g_t  (cross-base)
                if skip_h:
                    nc.vector.scalar_tensor_tensor(
                        cs, fi[64:128], 1.0, og[64:128], ALU.add, ALU.mult)
                else:
                    t0 = t0p.tile([64, 512], F32, tag="t0")
                    nc.vector.scalar_tensor_tensor(
                        t0[:], fi[0:64], 1.0, cs, ALU.add, ALU.mult)
                    t1 = t1p.tile([64, 512], MMDT, tag="t1")
                    nc.vector.scalar_tensor_tensor(
                        t1[:], fi[64:128], 1.0, og[64:128],
                        ALU.add, ALU.mult)
                    nc.vector.scalar_tensor_tensor(
                        cs, t0[:], 0.5, t1[:], ALU.mult, ALU.add)
                # tail (tanh(c), h-write, shifts) is emitted one tile
                # later by gate_tail so the ACT queue never head-of-line
                # blocks on the DVE c-update chain
                return (R_w, R2_w, r0, og)

            def gate_tail(st):
                R_w, R2_w, r0, og = st
                cs = c_t[:, r0 * 64:r0 * 64 + 512]
                th = thp.tile([64, 512], MMDT, tag="th")
                nc.scalar.activation(th[:], cs, ACT.Tanh, scale=0.5)
                # h_stored = 2h = tanh(c)*(o_t+1) -> both buffers' lower
                thv = th[:].rearrange("p (r c) -> p r c", c=64)
                ogv = og[0:64].rearrange("p (r c) -> p r c", c=64)
                nc.vector.scalar_tensor_tensor(
                    gview(R_w, 0, 64, LEAD + (r0 + 1) * PW + 1),
                    ogv, 1.0, thv, ALU.add, ALU.mult)
                nc.vector.scalar_tensor_tensor(
                    gview(R2_w, 0, 64, LEAD + (r0 + 1) * PW + 1),
                    ogv, 1.0, thv, ALU.add, ALU.mult)
                # col-shifted copy (+2) into R upper
                nc.sync.dma_start(
                    gview(R_w, 64, 128, LEAD + (r0 + 1) * PW - 1),
                    gview(R_w, 0, 64, LEAD + (r0 + 1) * PW + 1))
                # row-shifted copy (+1) into R2 upper
                nc.sync.dma_start(
                    gview(R2_w, 64, 128, LEAD + r0 * PW + 1),
                    gview(R_w, 0, 64, LEAD + (r0 + 1) * PW + 1))

            pend_tail = None
            x2_cur = emit_x2col(1)
            for s in range(1, NSTEPS + 1):
                ph = "e" if s <= T else "d"
                rp = regions[s - 1]
                ntiles = rp // 8
                if s % 2 == 0:  # read buffers written at s-1
                    R_r, R_w, R2_r, R2_w = rrA, rrB, r2A, r2B
                else:
                    R_r, R_w, R2_r, R2_w = rrB, rrA, r2B, r2A

                x2v = x2_cur[:].rearrange("p (r c) -> p r c", c=PW)
                if s < NSTEPS:
                    x2_next = emit_x2col(s + 1)  # prefetch on gpsimd queue

                for n in range(ntiles):
                    st = gate_block(s, ph, R_r, R2_r, R_w, R2_w, x2v, 8 * n)
                    if pend_tail is not None:
                        gate_tail(pend_tail)
                    pend_tail = st
                    # prev decoder step's out conv, interleaved between
                    # gate tiles so its PSUM/ACT deps never stall the PE
                    if s > T + 1 and n < 4:
                        emit_outconv1(s - 1, R_r, R2_r, n)

                if s < NSTEPS:
                    x2_cur = x2_next

            # out conv for the final decoder step (NSTEPS even -> B buffer)
            if pend_tail is not None:
                gate_tail(pend_tail)
            for n2 in range(4):
                emit_outconv1(NSTEPS, rrB, r2B, n2)

    nc.compile()
    return nc


def _prep_core_inputs(core, enc_in, dec_in, enc_W, enc_b, dec_W, dec_b,
                      out_W, out_b, use_bf16=True):
    import ml_dtypes
    mm_np = ml_dtypes.bfloat16 if use_bf16 else np.float32
    b, half = core // 2, core % 2
    # gate permutation: [f, i, o, g]
    perm = np.concatenate([np.arange(0, 128), np.arange(192, 256),
                           np.arange(128, 192)])

    def prep_x(x):
        x = x[b]  # [T, F, 64, 64]
        if half:
            x = x[:, :, ::-1, :]
        xp = np.zeros((T, F, PW, PW), np.float32)
        xp[:, :, 1:65, 1:65] = x
        return np.ascontiguousarray(xp)

    def prep_gateW(W, bias):
        Wf = W[:, :, ::-1, :] if half else W
        Wp = np.ascontiguousarray(Wf[perm])  # [256, 72, 3, 3]
        bp = bias[perm].astype(np.float32)
        # x part: rows (ky*3+kx)*8+ic, zero-padded to K=128
        lx = np.zeros((128, 256), np.float32)
        lx[0:72] = Wp[:, :F].transpose(2, 3, 1, 0).reshape(72, 256)
        # h-tap weights halved: stored h carries a 2x factor
        lp = [0.5 * np.concatenate(
            [Wp[:, F:, k, 0].T, Wp[:, F:, k, 2].T], axis=0)
            for k in range(3)]  # [128, 256]
        lma = 0.5 * np.concatenate([Wp[:, F:, 0, 1].T, Wp[:, F:, 1, 1].T],
                                   axis=0)  # [128, 256]
        lmb = np.zeros((128, 256), np.float32)
        lmb[0:64] = 0.5 * Wp[:, F:, 2, 1].T
        # tanh-form biases: f,i,o halves get b/2 (sigmoid via tanh(z/2))
        b0 = 0.5 * bp[0:128]
        b1 = np.concatenate([0.5 * bp[128:192], bp[192:256]])
        return (np.ascontiguousarray(lx),
                [np.ascontiguousarray(a) for a in lp],
                np.ascontiguousarray(lma), lmb,
                np.ascontiguousarray(b0.reshape(128, 1)),
                np.ascontiguousarray(b1.reshape(128, 1)))

    ex, ep, ema, emb, eb0, eb1 = prep_gateW(enc_W, enc_b)
    dx, dp, dma_, dmb, db0, db1 = prep_gateW(dec_W, dec_b)
    oWf = out_W[:, :, ::-1, :] if half else out_W
    op = [np.ascontiguousarray(0.5 * np.concatenate(
        [oWf[:, :, k, 0].T, oWf[:, :, k, 2].T], axis=0).astype(np.float32))
        for k in range(3)]
    oma = np.ascontiguousarray(0.5 * np.concatenate(
        [oWf[:, :, 0, 1].T, oWf[:, :, 1, 1].T], axis=0))
    omb = np.zeros((128, 8), np.float32)
    omb[0:64] = 0.5 * oWf[:, :, 2, 1].T
    scl = np.concatenate([np.full(64, 0.5, np.float32),
                          np.full(64, 1.0, np.float32)]).reshape(128, 1)

    m = {"xe": prep_x(enc_in), "xd": prep_x(dec_in),
         "w_ex": ex, "w_dx": dx,
         "w_ema": ema, "w_emb": emb, "w_dma": dma_, "w_dmb": dmb,
         "w_oma": oma, "w_omb": omb, "scl": scl,
         "b_e0": eb0, "b_e1": eb1, "b_d0": db0, "b_d1": db1,
         "b_o": np.ascontiguousarray(out_b.reshape(8, 1).astype(np.float32))}
    for k in range(3):
        m[f"w_ep{k}"] = ep[k]
        m[f"w_dp{k}"] = dp[k]
        m[f"w_op{k}"] = op[k]
    f32_keys = {"b_e0", "b_e1", "b_d0", "b_d1", "b_o", "scl"}
    return {k: np.ascontiguousarray(np.asarray(
        v, np.float32 if k in f32_keys else mm_np)) for k, v in m.items()}


def _install_trace_hook():
    """Shim antenv.axon_hooks for NTFF profiling (dev only)."""
    import contextlib
    import ctypes
    import types

    so = "/opt/axon/libaxon_pjrt.so"
    if "antenv.axon_hooks" in sys.modules or not os.path.exists(so):
        return
    lib = ctypes.CDLL(so)
    if not hasattr(lib, "axon_start_nrt_profile"):
        return
    lib.axon_start_nrt_profile.argtypes = [ctypes.POINTER(ctypes.c_int64),
                                           ctypes.c_size_t]
    lib.axon_start_nrt_profile.restype = ctypes.c_int64
    lib.axon_stop_nrt_profile.argtypes = [ctypes.c_char_p]
    lib.axon_stop_nrt_profile.restype = ctypes.c_int64

    def _mk():
        @contextlib.contextmanager
        def _hook(output_dir, device_ids):
            import jax
            jax.devices()
            if device_ids:
                ids = (ctypes.c_int64 * len(device_ids))(*device_ids)
                rc = lib.axon_start_nrt_profile(ids, len(device_ids))
            else:
                rc = lib.axon_start_nrt_profile(None, 0)
            if rc != 0:
                raise RuntimeError(f"axon_start_nrt_profile rc={rc}")
            try:
                yield
            finally:
                lib.axon_stop_nrt_profile(str(output_dir).encode())
        return _hook

    mod = types.ModuleType("antenv.axon_hooks")
    mod.get_axon_ntff_profile_hook = _mk
    sys.modules["antenv.axon_hooks"] = mod


def kernel(enc_in, dec_in, enc_W, enc_b, dec_W, dec_b, out_W, out_b):
    from concourse.bass_utils import run_bass_kernel_spmd

    trace = os.environ.get("KERNEL_TRACE", "") == "1"
    if trace:
        _install_trace_hook()

    use_bf16 = os.environ.get("KERNEL_DTYPE", "bf16") != "f32r"
    if "nc" not in _CACHE:
        _CACHE["nc"] = _build_program(use_bf16)
    nc = _CACHE["nc"]

    args = (np.asarray(enc_in, np.float32), np.asarray(dec_in, np.float32),
            np.asarray(enc_W, np.float32), np.asarray(enc_b, np.float32),
            np.asarray(dec_W, np.float32), np.asarray(dec_b, np.float32),
            np.asarray(out_W, np.float32), np.asarray(out_b, np.float32))
    in_maps = [_prep_core_inputs(c, *args, use_bf16=use_bf16)
               for c in range(NCORES)]

    res = run_bass_kernel_spmd(nc, in_maps, list(range(NCORES)), trace=trace)
    if trace:
        _CACHE["exec_time_ns"] = res.exec_time_ns

    B = enc_in.shape[0]
    out = np.empty((B, T, F, HS, WS), np.float32)
    for c in range(NCORES):
        b, half = c // 2, c % 2
        yc = res.results[c]["y"]  # [T, F, 32, 64]
        if half:
            out[b, :, :, 32:64, :] = yc[:, :, ::-1, :]
        else:
            out[b, :, :, 0:32, :] = yc
    return out
